# revision 3
# baseline (speedup 1.0000x reference)
"""Trainium2 Bass kernel for nn_BoundleAdjustment.

Observations are data-parallel sharded across the 8 NeuronCores (M/8 each).
The host stages per-observation records (pose row, patch row, target, weight)
into 14 planar [128, 2048] f32 streams per core; the device kernel streams
them through SBUF and performs the full math on-chip: quaternion
normalization (2/|q|^2 scaling), SE3 rotation+translation (two cross
products), cartesian->polar (Square/Sqrt/Arctan/Sign on the Scalar engine),
and the weighted residual.

Engine balance: elementwise tensor ops are split between the Vector engine
and GPSIMD (whole ops, chosen so both engines stay busy); transcendentals
run on the Scalar engine; all three divisions share a single DVE reciprocal
via the product trick 1/a = (b*c)/(a*b*c); DMA issue lives on the SP queue.
"""

import numpy as np

M = 2097152
NCORES = 8
N = M // NCORES          # 262144 obs per core
P = 128
COLS = N // P            # 2048 obs per partition
CC = 1024                # obs columns per chunk
NCH = COLS // CC         # 2 chunks
NPLANES = 14             # tx ty tz qx qy qz qw px py pz X Y Z w
PI = float(np.pi)

_CACHE = {}


def _build():
    import concourse.bass as bass
    import concourse.tile as tile
    from concourse import bacc, mybir

    nc = bacc.Bacc("TRN2", target_bir_lowering=False, debug=False,
                   num_devices=NCORES)
    f32 = mybir.dt.float32
    AF = mybir.ActivationFunctionType
    OP = mybir.AluOpType
    in_d = nc.declare_dram_parameter("in", [NPLANES, P, COLS], f32,
                                     isOutput=False)
    out_d = nc.declare_dram_parameter("out", [3, P, COLS], f32, isOutput=True)

    with tile.TileContext(nc) as tc:
        with tc.tile_pool(name="inp", bufs=2) as inp, \
             tc.tile_pool(name="tmpp", bufs=20) as tmpp:
            for ch in range(NCH):
                ins = []
                for k in range(NPLANES):
                    t = inp.tile([P, CC], f32, tag=f"in{k}", name=f"in{k}_{ch}")
                    nc.sync.dma_start(t[:], in_d[k, :, bass.ts(ch, CC)])
                    ins.append(t)
                (tx, ty, tz, qx, qy, qz, qw, px, py, pz, X, Y, Z, W) = ins

                cnt = [0]

                def T(name=None):
                    cnt[0] += 1
                    return tmpp.tile([P, CC], f32, tag="tmp",
                                     name=f"tmp{ch}_{cnt[0]}")

                def tt(eng, a, b, op):
                    d = T()
                    eng.tensor_tensor(out=d[:], in0=a[:], in1=b[:], op=op)
                    return d

                vec, pool, act = nc.vector, nc.gpsimd, nc.scalar

                def sq(a):
                    d = T()
                    act.activation(d[:], a[:], AF.Square)
                    return d

                # ---- s = |q|^2 (squares on ACT, adds on Pool) ----
                sq0, sq1, sq2, sq3 = sq(qx), sq(qy), sq(qz), sq(qw)
                s01 = tt(pool, sq0, sq1, OP.add)
                s23 = tt(pool, sq2, sq3, OP.add)
                s = tt(pool, s01, s23, OP.add)
                d1 = T()
                vec.tensor_scalar(out=d1[:], in0=s[:], scalar1=0.5,
                                  scalar2=None, op0=OP.mult)   # d1 = s/2

                # ---- uv = qv x pts (Pool) ----
                def cross(engm, a1, a2, b1, b2):
                    m1 = tt(engm, a1, a2, OP.mult)
                    m2 = tt(engm, b1, b2, OP.mult)
                    return tt(engm, m1, m2, OP.subtract)

                uvx = cross(pool, qy, pz, qz, py)
                uvy = cross(pool, qz, px, qx, pz)
                uvz = cross(pool, qx, py, qy, px)

                # ---- wv = qw*uv + qv x uv (DVE) ----
                def wcomp(uv_a, q_b, uv_b, q_c, uv_c):
                    m0 = tt(vec, qw, uv_a, OP.mult)
                    m1 = tt(vec, q_b, uv_b, OP.mult)
                    a0 = tt(vec, m0, m1, OP.add)
                    m2 = tt(vec, q_c, uv_c, OP.mult)
                    return tt(vec, a0, m2, OP.subtract)

                wx = wcomp(uvx, qy, uvz, qz, uvy)
                wy = wcomp(uvy, qz, uvx, qx, uvz)
                wz = wcomp(uvz, qx, uvy, qy, uvx)

                # ---- u = 1/d1 deferred via product trick below; first need
                # rx,ry,rz which need u. So compute u with its own pieces:
                # we need 1/d1, 1/rx, 1/rho2 later; rx depends on u -> the
                # product trick can only merge 1/rx, 1/rho2. Do recip(d1) now.
                u = T()
                vec.reciprocal(u[:], d1[:])              # u = 2/|q|^2

                # ---- r = pts + u*wv + t (mult on DVE, adds on Pool) ----
                def rcomp(wc, pc, tc_):
                    m = tt(vec, wc, u, OP.mult)
                    a = tt(pool, m, pc, OP.add)
                    return tt(pool, a, tc_, OP.add)

                rx = rcomp(wx, px, tx)
                ry = rcomp(wy, py, ty)
                rz = rcomp(wz, pz, tz)

                # ---- polar ----
                sqx_, sqy_, sqz_ = sq(rx), sq(ry), sq(rz)
                rho2 = tt(vec, sqx_, sqy_, OP.add)
                r2 = tt(vec, rho2, sqz_, OP.add)
                rng = T(); act.activation(rng[:], r2[:], AF.Sqrt)
                rho = T(); act.activation(rho[:], rho2[:], AF.Sqrt)

                # merged reciprocal for 1/rx and 1/rho2:
                # Pq = rx*rho2 ; ip = 1/Pq ; irx = ip*rho2 ; irho2 = ip*rx
                Pq = tt(vec, rx, rho2, OP.mult)
                ip = T()
                vec.reciprocal(ip[:], Pq[:])
                irx = tt(vec, ip, rho2, OP.mult)
                irho2 = tt(vec, ip, rx, OP.mult)
                irho = tt(vec, rho, irho2, OP.mult)      # 1/rho = rho/rho2

                # az = atan(ry/rx) + sign(ry)*pi*(rx<0)
                a1 = tt(vec, ry, irx, OP.mult)
                az0 = T(); act.activation(az0[:], a1[:], AF.Arctan)
                sg = T(); act.activation(sg[:], ry[:], AF.Sign)
                msk = T()
                vec.tensor_scalar(out=msk[:], in0=rx[:], scalar1=0.0,
                                  scalar2=PI, op0=OP.is_lt, op1=OP.mult)
                corr = tt(vec, msk, sg, OP.mult)
                az = tt(vec, az0, corr, OP.add)

                # el = atan(rz/rho)
                e1 = tt(vec, rz, irho, OP.mult)
                el = T(); act.activation(el[:], e1[:], AF.Arctan)

                # residual = (proj - target) * w  (subs on Pool, muls on DVE)
                for (pcomp, tgt, idx) in ((rng, X, 0), (az, Y, 1), (el, Z, 2)):
                    dsub = tt(pool, pcomp, tgt, OP.subtract)
                    o = tt(vec, dsub, W, OP.mult)
                    nc.sync.dma_start(out_d[idx, :, bass.ts(ch, CC)], o[:])
    nc.finalize()
    return nc


def _get_nc():
    if "nc" not in _CACHE:
        _CACHE["nc"] = _build()
    return _CACHE["nc"]


def _stage_inputs(poses, patch_coords, elevation_angle, poses_idx, patch_idx,
                  target_coords, weights):
    poses = np.asarray(poses, dtype=np.float32)
    patch_coords = np.asarray(patch_coords, dtype=np.float32)
    elevation_angle = np.asarray(elevation_angle, dtype=np.float32)
    target_coords = np.asarray(target_coords, dtype=np.float32)
    weights = np.asarray(weights, dtype=np.float32)
    pid = np.asarray(poses_idx).astype(np.int64)
    qid = np.asarray(patch_idx).astype(np.int64)

    p7 = poses[pid]                                           # [M, 7]
    pts = np.concatenate(
        [patch_coords[qid], elevation_angle[qid]], axis=1)    # [M, 3]
    big = np.concatenate([p7, pts, target_coords, weights], axis=1)  # [M,14]
    return np.ascontiguousarray(
        big.reshape(NCORES, P, COLS, NPLANES).transpose(0, 3, 1, 2))


def kernel(poses, patch_coords, elevation_angle, poses_idx, patch_idx,
           target_coords, weights):
    from concourse.bass_utils import run_bass_kernel_spmd

    big = _stage_inputs(poses, patch_coords, elevation_angle, poses_idx,
                        patch_idx, target_coords, weights)
    nc = _get_nc()
    in_maps = [{"in": big[c]} for c in range(NCORES)]
    res = run_bass_kernel_spmd(nc, in_maps, list(range(NCORES)))
    out = np.stack([res.results[c]["out"] for c in range(NCORES)])
    # [NCORES, 3, P, COLS] -> [M, 3]
    return np.ascontiguousarray(
        out.transpose(0, 2, 3, 1).reshape(M, 3)).astype(np.float32)


# revision 4
# speedup vs baseline: 1.1034x; 1.1034x over previous
"""Trainium2 Bass kernel for nn_BoundleAdjustment.

Observations are data-parallel sharded across the 8 NeuronCores (M/8 each).
The host stages per-observation records (pose row, patch row, target, weight)
into 14 planar [128, 2048] f32 streams per core; the device kernel streams
them through SBUF and performs the full math on-chip: quaternion
normalization (2/|q|^2 scaling), SE3 rotation+translation (two cross
products), cartesian->polar (Square/Sqrt/Arctan/Sign on the Scalar engine),
and the weighted residual.

Engine balance: elementwise tensor ops are split between the Vector engine
and GPSIMD (whole ops, chosen so both engines stay busy); transcendentals
run on the Scalar engine; all three divisions share a single DVE reciprocal
via the product trick 1/a = (b*c)/(a*b*c); DMA issue lives on the SP queue.
"""

import numpy as np

M = 2097152
NCORES = 8
N = M // NCORES          # 262144 obs per core
P = 128
COLS = N // P            # 2048 obs per partition
CC = 1024                # obs columns per chunk
NCH = COLS // CC         # 2 chunks
NPLANES = 14             # tx ty tz qx qy qz qw px py pz X Y Z w
PI = float(np.pi)

_CACHE = {}


def _build():
    import concourse.bass as bass
    import concourse.tile as tile
    from concourse import bacc, mybir

    nc = bacc.Bacc("TRN2", target_bir_lowering=False, debug=False,
                   num_devices=NCORES)
    f32 = mybir.dt.float32
    AF = mybir.ActivationFunctionType
    OP = mybir.AluOpType
    in_d = nc.declare_dram_parameter("in", [NPLANES, P, COLS], f32,
                                     isOutput=False)
    out_d = nc.declare_dram_parameter("out", [3, P, COLS], f32, isOutput=True)

    with tile.TileContext(nc) as tc:
        with tc.tile_pool(name="inp", bufs=2) as inp, \
             tc.tile_pool(name="tmpp", bufs=20) as tmpp:
            for ch in range(NCH):
                ins = []
                for k in range(NPLANES):
                    t = inp.tile([P, CC], f32, tag=f"in{k}", name=f"in{k}_{ch}")
                    nc.sync.dma_start(t[:], in_d[k, :, bass.ts(ch, CC)])
                    ins.append(t)
                (tx, ty, tz, qx, qy, qz, qw, px, py, pz, X, Y, Z, W) = ins

                cnt = [0]

                def T(name=None):
                    cnt[0] += 1
                    return tmpp.tile([P, CC], f32, tag="tmp",
                                     name=f"tmp{ch}_{cnt[0]}")

                def tt(eng, a, b, op):
                    d = T()
                    eng.tensor_tensor(out=d[:], in0=a[:], in1=b[:], op=op)
                    return d

                vec, pool, act = nc.vector, nc.gpsimd, nc.scalar

                def sq(a):
                    d = T()
                    act.activation(d[:], a[:], AF.Square)
                    return d

                # ---- s = |q|^2 (squares on ACT, adds on Pool) ----
                sq0, sq1, sq2, sq3 = sq(qx), sq(qy), sq(qz), sq(qw)
                s01 = tt(vec, sq0, sq1, OP.add)
                s23 = tt(vec, sq2, sq3, OP.add)
                s = tt(vec, s01, s23, OP.add)
                d1 = T()
                vec.tensor_scalar(out=d1[:], in0=s[:], scalar1=0.5,
                                  scalar2=None, op0=OP.mult)   # d1 = s/2

                # ---- uv = qv x pts (Pool) ----
                def cross(engm, a1, a2, b1, b2):
                    m1 = tt(engm, a1, a2, OP.mult)
                    m2 = tt(engm, b1, b2, OP.mult)
                    return tt(engm, m1, m2, OP.subtract)

                uvx = cross(pool, qy, pz, qz, py)
                uvy = cross(pool, qz, px, qx, pz)
                uvz = cross(pool, qx, py, qy, px)

                # ---- wv = qw*uv + qv x uv (DVE) ----
                def wcomp(uv_a, q_b, uv_b, q_c, uv_c):
                    m0 = tt(vec, qw, uv_a, OP.mult)
                    m1 = tt(vec, q_b, uv_b, OP.mult)
                    a0 = tt(vec, m0, m1, OP.add)
                    m2 = tt(vec, q_c, uv_c, OP.mult)
                    return tt(vec, a0, m2, OP.subtract)

                wx = wcomp(uvx, qy, uvz, qz, uvy)
                wy = wcomp(uvy, qz, uvx, qx, uvz)
                wz = wcomp(uvz, qx, uvy, qy, uvx)

                # ---- u = 1/d1 deferred via product trick below; first need
                # rx,ry,rz which need u. So compute u with its own pieces:
                # we need 1/d1, 1/rx, 1/rho2 later; rx depends on u -> the
                # product trick can only merge 1/rx, 1/rho2. Do recip(d1) now.
                u = T()
                vec.reciprocal(u[:], d1[:])              # u = 2/|q|^2

                # ---- r = pts + u*wv + t (mult on DVE, adds on Pool) ----
                def rcomp(wc, pc, tc_):
                    m = tt(vec, wc, u, OP.mult)
                    a = tt(vec, m, pc, OP.add)
                    return tt(vec, a, tc_, OP.add)

                rx = rcomp(wx, px, tx)
                ry = rcomp(wy, py, ty)
                rz = rcomp(wz, pz, tz)

                # ---- polar ----
                sqx_, sqy_, sqz_ = sq(rx), sq(ry), sq(rz)
                rho2 = tt(vec, sqx_, sqy_, OP.add)
                r2 = tt(vec, rho2, sqz_, OP.add)
                rng = T(); act.activation(rng[:], r2[:], AF.Sqrt)
                rho = T(); act.activation(rho[:], rho2[:], AF.Sqrt)

                # merged reciprocal for 1/rx and 1/rho2:
                # Pq = rx*rho2 ; ip = 1/Pq ; irx = ip*rho2 ; irho2 = ip*rx
                Pq = tt(vec, rx, rho2, OP.mult)
                ip = T()
                vec.reciprocal(ip[:], Pq[:])
                irx = tt(vec, ip, rho2, OP.mult)
                irho2 = tt(vec, ip, rx, OP.mult)
                irho = tt(vec, rho, irho2, OP.mult)      # 1/rho = rho/rho2

                # az = atan(ry/rx) + sign(ry)*pi*(rx<0)
                a1 = tt(vec, ry, irx, OP.mult)
                az0 = T(); act.activation(az0[:], a1[:], AF.Arctan)
                sg = T(); act.activation(sg[:], ry[:], AF.Sign)
                msk = T()
                vec.tensor_scalar(out=msk[:], in0=rx[:], scalar1=0.0,
                                  scalar2=PI, op0=OP.is_lt, op1=OP.mult)
                corr = tt(vec, msk, sg, OP.mult)
                az = tt(vec, az0, corr, OP.add)

                # el = atan(rz/rho)
                e1 = tt(vec, rz, irho, OP.mult)
                el = T(); act.activation(el[:], e1[:], AF.Arctan)

                # residual = (proj - target) * w  (subs on Pool, muls on DVE)
                for (pcomp, tgt, idx) in ((rng, X, 0), (az, Y, 1), (el, Z, 2)):
                    dsub = tt(vec, pcomp, tgt, OP.subtract)
                    o = tt(vec, dsub, W, OP.mult)
                    nc.sync.dma_start(out_d[idx, :, bass.ts(ch, CC)], o[:])
    nc.finalize()
    return nc


def _get_nc():
    if "nc" not in _CACHE:
        _CACHE["nc"] = _build()
    return _CACHE["nc"]


def _stage_inputs(poses, patch_coords, elevation_angle, poses_idx, patch_idx,
                  target_coords, weights):
    poses = np.asarray(poses, dtype=np.float32)
    patch_coords = np.asarray(patch_coords, dtype=np.float32)
    elevation_angle = np.asarray(elevation_angle, dtype=np.float32)
    target_coords = np.asarray(target_coords, dtype=np.float32)
    weights = np.asarray(weights, dtype=np.float32)
    pid = np.asarray(poses_idx).astype(np.int64)
    qid = np.asarray(patch_idx).astype(np.int64)

    p7 = poses[pid]                                           # [M, 7]
    pts = np.concatenate(
        [patch_coords[qid], elevation_angle[qid]], axis=1)    # [M, 3]
    big = np.concatenate([p7, pts, target_coords, weights], axis=1)  # [M,14]
    return np.ascontiguousarray(
        big.reshape(NCORES, P, COLS, NPLANES).transpose(0, 3, 1, 2))


def kernel(poses, patch_coords, elevation_angle, poses_idx, patch_idx,
           target_coords, weights):
    from concourse.bass_utils import run_bass_kernel_spmd

    big = _stage_inputs(poses, patch_coords, elevation_angle, poses_idx,
                        patch_idx, target_coords, weights)
    nc = _get_nc()
    in_maps = [{"in": big[c]} for c in range(NCORES)]
    res = run_bass_kernel_spmd(nc, in_maps, list(range(NCORES)))
    out = np.stack([res.results[c]["out"] for c in range(NCORES)])
    # [NCORES, 3, P, COLS] -> [M, 3]
    return np.ascontiguousarray(
        out.transpose(0, 2, 3, 1).reshape(M, 3)).astype(np.float32)


# revision 5
# speedup vs baseline: 1.2321x; 1.1167x over previous
"""Trainium2 Bass kernel for nn_BoundleAdjustment.

Observations are data-parallel sharded across the 8 NeuronCores (M/8 each).
The host stages per-observation records (pose row, patch row, target, weight)
into 14 planar [128, 2048] f32 streams per core; the device kernel streams
them through SBUF and performs the full math on-chip: quaternion
normalization (2/|q|^2 scaling), SE3 rotation+translation (two cross
products), cartesian->polar (Square/Sqrt/Arctan/Sign on the Scalar engine),
and the weighted residual.

Engine balance: elementwise tensor ops are split between the Vector engine
and GPSIMD (whole ops, chosen so both engines stay busy); transcendentals
run on the Scalar engine; all three divisions share a single DVE reciprocal
via the product trick 1/a = (b*c)/(a*b*c); DMA issue lives on the SP queue.
"""

import numpy as np

M = 2097152
NCORES = 8
N = M // NCORES          # 262144 obs per core
P = 128
COLS = N // P            # 2048 obs per partition
CC = 1024                # obs columns per chunk
NCH = COLS // CC         # 2 chunks
NPLANES = 14             # tx ty tz qx qy qz qw px py pz X Y Z w
PI = float(np.pi)

_CACHE = {}


def _build():
    import concourse.bass as bass
    import concourse.tile as tile
    from concourse import bacc, mybir

    nc = bacc.Bacc("TRN2", target_bir_lowering=False, debug=False,
                   num_devices=NCORES)
    f32 = mybir.dt.float32
    AF = mybir.ActivationFunctionType
    OP = mybir.AluOpType
    in_d = nc.declare_dram_parameter("in", [NPLANES, P, COLS], f32,
                                     isOutput=False)
    out_d = nc.declare_dram_parameter("out", [3, P, COLS], f32, isOutput=True)

    with tile.TileContext(nc) as tc:
        with tc.tile_pool(name="inp", bufs=2) as inp, \
             tc.tile_pool(name="tmpp", bufs=20) as tmpp:
            for ch in range(NCH):
                ins = []
                for k in range(NPLANES):
                    t = inp.tile([P, CC], f32, tag=f"in{k}", name=f"in{k}_{ch}")
                    nc.sync.dma_start(t[:], in_d[k, :, bass.ts(ch, CC)])
                    ins.append(t)
                (tx, ty, tz, qx, qy, qz, qw, px, py, pz, X, Y, Z, W) = ins

                cnt = [0]

                def T(name=None):
                    cnt[0] += 1
                    return tmpp.tile([P, CC], f32, tag="tmp",
                                     name=f"tmp{ch}_{cnt[0]}")

                def tt(eng, a, b, op):
                    d = T()
                    eng.tensor_tensor(out=d[:], in0=a[:], in1=b[:], op=op)
                    return d

                vec, pool, act = nc.vector, nc.gpsimd, nc.scalar

                def sq(a):
                    d = T()
                    act.activation(d[:], a[:], AF.Square)
                    return d

                # ---- s = |q|^2 (squares on ACT, adds on Pool) ----
                sq0, sq1, sq2, sq3 = sq(qx), sq(qy), sq(qz), sq(qw)
                s01 = tt(vec, sq0, sq1, OP.add)
                s23 = tt(vec, sq2, sq3, OP.add)
                s = tt(vec, s01, s23, OP.add)
                d1 = T()
                vec.tensor_scalar(out=d1[:], in0=s[:], scalar1=0.5,
                                  scalar2=None, op0=OP.mult)   # d1 = s/2

                # ---- uv = qv x pts (Pool) ----
                def cross(engm, a1, a2, b1, b2):
                    m1 = tt(engm, a1, a2, OP.mult)
                    m2 = tt(engm, b1, b2, OP.mult)
                    return tt(engm, m1, m2, OP.subtract)

                uvx = cross(vec, qy, pz, qz, py)
                uvy = cross(vec, qz, px, qx, pz)
                uvz = cross(vec, qx, py, qy, px)

                # ---- wv = qw*uv + qv x uv (DVE) ----
                def wcomp(uv_a, q_b, uv_b, q_c, uv_c):
                    m0 = tt(vec, qw, uv_a, OP.mult)
                    m1 = tt(vec, q_b, uv_b, OP.mult)
                    a0 = tt(vec, m0, m1, OP.add)
                    m2 = tt(vec, q_c, uv_c, OP.mult)
                    return tt(vec, a0, m2, OP.subtract)

                wx = wcomp(uvx, qy, uvz, qz, uvy)
                wy = wcomp(uvy, qz, uvx, qx, uvz)
                wz = wcomp(uvz, qx, uvy, qy, uvx)

                # ---- u = 1/d1 deferred via product trick below; first need
                # rx,ry,rz which need u. So compute u with its own pieces:
                # we need 1/d1, 1/rx, 1/rho2 later; rx depends on u -> the
                # product trick can only merge 1/rx, 1/rho2. Do recip(d1) now.
                u = T()
                vec.reciprocal(u[:], d1[:])              # u = 2/|q|^2

                # ---- r = pts + u*wv + t (mult on DVE, adds on Pool) ----
                def rcomp(wc, pc, tc_):
                    m = tt(vec, wc, u, OP.mult)
                    a = tt(vec, m, pc, OP.add)
                    return tt(vec, a, tc_, OP.add)

                rx = rcomp(wx, px, tx)
                ry = rcomp(wy, py, ty)
                rz = rcomp(wz, pz, tz)

                # ---- polar ----
                sqx_, sqy_, sqz_ = sq(rx), sq(ry), sq(rz)
                rho2 = tt(vec, sqx_, sqy_, OP.add)
                r2 = tt(vec, rho2, sqz_, OP.add)
                rng = T(); act.activation(rng[:], r2[:], AF.Sqrt)
                rho = T(); act.activation(rho[:], rho2[:], AF.Sqrt)

                # merged reciprocal for 1/rx and 1/rho2:
                # Pq = rx*rho2 ; ip = 1/Pq ; irx = ip*rho2 ; irho2 = ip*rx
                Pq = tt(vec, rx, rho2, OP.mult)
                ip = T()
                vec.reciprocal(ip[:], Pq[:])
                irx = tt(vec, ip, rho2, OP.mult)
                irho2 = tt(vec, ip, rx, OP.mult)
                irho = tt(vec, rho, irho2, OP.mult)      # 1/rho = rho/rho2

                # az = atan(ry/rx) + sign(ry)*pi*(rx<0)
                a1 = tt(vec, ry, irx, OP.mult)
                az0 = T(); act.activation(az0[:], a1[:], AF.Arctan)
                sg = T(); act.activation(sg[:], ry[:], AF.Sign)
                msk = T()
                vec.tensor_scalar(out=msk[:], in0=rx[:], scalar1=0.0,
                                  scalar2=PI, op0=OP.is_lt, op1=OP.mult)
                corr = tt(vec, msk, sg, OP.mult)
                az = tt(vec, az0, corr, OP.add)

                # el = atan(rz/rho)
                e1 = tt(vec, rz, irho, OP.mult)
                el = T(); act.activation(el[:], e1[:], AF.Arctan)

                # residual = (proj - target) * w  (subs on Pool, muls on DVE)
                for (pcomp, tgt, idx) in ((rng, X, 0), (az, Y, 1), (el, Z, 2)):
                    dsub = tt(vec, pcomp, tgt, OP.subtract)
                    o = tt(vec, dsub, W, OP.mult)
                    nc.sync.dma_start(out_d[idx, :, bass.ts(ch, CC)], o[:])
    nc.finalize()
    return nc


def _get_nc():
    if "nc" not in _CACHE:
        _CACHE["nc"] = _build()
    return _CACHE["nc"]


def _stage_inputs(poses, patch_coords, elevation_angle, poses_idx, patch_idx,
                  target_coords, weights):
    poses = np.asarray(poses, dtype=np.float32)
    patch_coords = np.asarray(patch_coords, dtype=np.float32)
    elevation_angle = np.asarray(elevation_angle, dtype=np.float32)
    target_coords = np.asarray(target_coords, dtype=np.float32)
    weights = np.asarray(weights, dtype=np.float32)
    pid = np.asarray(poses_idx).astype(np.int64)
    qid = np.asarray(patch_idx).astype(np.int64)

    p7 = poses[pid]                                           # [M, 7]
    pts = np.concatenate(
        [patch_coords[qid], elevation_angle[qid]], axis=1)    # [M, 3]
    big = np.concatenate([p7, pts, target_coords, weights], axis=1)  # [M,14]
    return np.ascontiguousarray(
        big.reshape(NCORES, P, COLS, NPLANES).transpose(0, 3, 1, 2))


def kernel(poses, patch_coords, elevation_angle, poses_idx, patch_idx,
           target_coords, weights):
    from concourse.bass_utils import run_bass_kernel_spmd

    big = _stage_inputs(poses, patch_coords, elevation_angle, poses_idx,
                        patch_idx, target_coords, weights)
    nc = _get_nc()
    in_maps = [{"in": big[c]} for c in range(NCORES)]
    res = run_bass_kernel_spmd(nc, in_maps, list(range(NCORES)))
    out = np.stack([res.results[c]["out"] for c in range(NCORES)])
    # [NCORES, 3, P, COLS] -> [M, 3]
    return np.ascontiguousarray(
        out.transpose(0, 2, 3, 1).reshape(M, 3)).astype(np.float32)
